# revision 22
# baseline (speedup 1.0000x reference)
"""Trainium2 Bass kernel for nn_CoAttention (hyperbolic co-attention).

Sharding: pure data parallel over batch B=16 -> 8 cores x 2 batches.
All per-batch intermediates stay on-chip; only [2,1024,128] inputs
stream in and the small outputs (As/Ac/co) stream out per core.

Layouts per batch (S = NS = NC = 1024, D = 128, Dp = 129):
  row-major [1024, d] tensors are stored as [128 part, 8 s_hi, d]
  (s = s_hi*128 + part), so matmul m-tiles are contiguous slices and
  per-row scalars live in [128, 8] "column" tiles.

Runtime constraints discovered by bisection on this axon runtime:
  - custom-DVE-table ops (tensor_tensor_reduce, reciprocal_approx_*)
    crash the device -> only native ISA ops are used.
  - gpsimd custom ops avoided (identity matrix shipped from host).

Math notes (vs reference.py):
  - sigmoid(x) = 1/(1+exp(-x))            (avoids the sigmoid ACT set)
  - all sqrt/rsqrt computed on the DVE via the 0x5f3759df exponent-seed
    + 3 Newton steps (fp32-exact, ~1.5e-7); the scalar engine then only
    needs the exp_and_others table set (exp/tanh/square/copy) -> no ACT
    table-set switching at all.
  - artanh(clip(norm)) == artanh(1-1e-5)  for the >=1-by-construction
    row norms -> constant C_ART.
  - mobius_scale/mobius_add/p2l folded into per-row scalar algebra.
"""
import sys

sys.path.insert(0, "/opt/trn_rl_repo")

from contextlib import ExitStack

import numpy as np

import concourse.bass as bass
import concourse.mybir as mybir
import concourse.tile as tile
from concourse import bacc
from concourse.bass_utils import run_bass_kernel_spmd

FP = mybir.dt.float32
I32 = mybir.dt.int32
AF = mybir.ActivationFunctionType
OP = mybir.AluOpType
AX = mybir.AxisListType

B, S, D = 16, 1024, 128
Dp = D + 1
SH = S // 128              # 8 row-tiles of 128
NCORES = 8
BPC = B // NCORES          # batches per core
SQK = float(np.sqrt(128.0))          # sqrt(K_CONV)
C_ART = float(np.arctanh(1.0 - 1e-5))
MINN = 1e-15
MAGIC = 0x5F3759DF
NEWTON = 2

# wide (1023/1024-long) row sums-of-squares on ACT (Square + accum_out)
ACT_ACCUM_OK = True


def _build(consts: dict):
    """consts: sl/ss/sc = exp(log_scale) floats."""
    nc = bacc.Bacc("TRN2", target_bir_lowering=False, debug=False)

    sr = nc.dram_tensor("sr", [BPC, S, D], FP, kind="ExternalInput")
    cr = nc.dram_tensor("cr", [BPC, S, D], FP, kind="ExternalInput")
    w_in = {}
    for nm in ("wl", "ws", "wc"):
        w_in[nm + "_sp"] = nc.dram_tensor(nm + "_sp", [D, Dp], FP, kind="ExternalInput")
        w_in[nm + "_tc"] = nc.dram_tensor(nm + "_tc", [2, Dp], FP, kind="ExternalInput")
    whs_d = nc.dram_tensor("whs_b", [128, SH * Dp], FP, kind="ExternalInput")
    whc_d = nc.dram_tensor("whc_b", [128, SH * Dp], FP, kind="ExternalInput")
    ident_d = nc.dram_tensor("ident_in", [128, 128], FP, kind="ExternalInput")

    as_d = nc.dram_tensor("as_out", [BPC, S], FP, kind="ExternalOutput")
    ac_d = nc.dram_tensor("ac_out", [BPC, S], FP, kind="ExternalOutput")
    co_d = nc.dram_tensor("co_out", [BPC, 2 * D + 1], FP, kind="ExternalOutput")

    sr_r = sr.rearrange("b (sh p) d -> b p sh d", p=128)
    cr_r = cr.rearrange("b (sh p) d -> b p sh d", p=128)
    as_r = as_d.rearrange("b (sh p) -> b p sh", p=128)
    ac_r = ac_d.rearrange("b (sh p) -> b p sh", p=128)
    co_r = co_d.rearrange("b (d one) -> b d one", one=1)

    with tile.TileContext(nc) as tc, ExitStack() as ctx:
        cpool = ctx.enter_context(tc.tile_pool(name="consts", bufs=1))
        big = ctx.enter_context(tc.tile_pool(name="big", bufs=1))
        mid = ctx.enter_context(tc.tile_pool(name="mid", bufs=1))
        cols = ctx.enter_context(tc.tile_pool(name="cols", bufs=2))
        ps_big = ctx.enter_context(tc.tile_pool(name="ps_big", bufs=2, space="PSUM"))
        ps_mid = ctx.enter_context(tc.tile_pool(name="ps_mid", bufs=2, space="PSUM"))
        ps_sm = ctx.enter_context(tc.tile_pool(name="ps_sm", bufs=2, space="PSUM"))

        # ---- constants ----
        ident = cpool.tile([128, 128], FP)
        nc.sync.dma_start(ident[:], ident_d[:])
        ones_col = cpool.tile([128, 1], FP)
        nc.vector.memset(ones_col[:], 1.0)
        ones_row = cpool.tile([1, 128], FP)
        nc.vector.memset(ones_row[:], 1.0)
        magic = cpool.tile([128, SH], I32)
        nc.vector.memset(magic[:], MAGIC)
        wsb = {}
        for nm in ("wl", "ws", "wc"):
            sp_t = cpool.tile([D, Dp], FP, name=nm + "_sp_sb")
            nc.sync.dma_start(sp_t[:], w_in[nm + "_sp"][:])
            tc_t = cpool.tile([2, Dp], FP, name=nm + "_tc_sb")
            nc.sync.dma_start(tc_t[:], w_in[nm + "_tc"][:])
            wsb[nm] = (sp_t, tc_t)
        whs_sb = cpool.tile([128, SH * Dp], FP)
        nc.sync.dma_start(whs_sb[:], whs_d[:])
        whc_sb = cpool.tile([128, SH * Dp], FP)
        nc.sync.dma_start(whc_sb[:], whc_d[:])

        # ---- helpers ----
        def col(tag):
            return cols.tile([128, SH], FP, name=tag, tag=tag)

        def ts(out, in0, s1, s2, op0, op1=None):
            if op1 is None:
                nc.vector.tensor_scalar(out, in0, s1, None, op0)
            else:
                nc.vector.tensor_scalar(out, in0, s1, s2, op0, op1)

        def recip(out, in_):
            nc.vector.reciprocal(out, in_)

        def act(out, in_, f, bias=0.0, scale=1.0, accum=None):
            nc.scalar.activation(out, in_, f, bias=bias, scale=scale,
                                 accum_out=accum)

        def rsqrt(out_ap, in_ap, p=128, w=SH):
            """out = 1/sqrt(in) on DVE. APs are [p, w]."""
            ti = cols.tile([128, SH], I32, name="rs_i", tag="rs_i")
            nc.vector.tensor_scalar(ti[0:p, 0:w], in_ap.bitcast(I32), 1, None,
                                    OP.logical_shift_right)
            nc.vector.tensor_sub(ti[0:p, 0:w], magic[0:p, 0:w], ti[0:p, 0:w])
            y = ti[0:p, 0:w].bitcast(FP)
            for it in range(NEWTON):
                a = cols.tile([128, SH], FP, name="rs_a", tag="rs_a")
                nc.vector.tensor_mul(a[0:p, 0:w], y, y)
                nc.vector.tensor_mul(a[0:p, 0:w], a[0:p, 0:w], in_ap)
                nc.vector.tensor_scalar(a[0:p, 0:w], a[0:p, 0:w], -0.5, 1.5,
                                        OP.mult, OP.add)
                if it == NEWTON - 1:
                    nc.vector.tensor_mul(out_ap, y, a[0:p, 0:w])
                else:
                    yn = cols.tile([128, SH], FP, name="rs_y", tag=f"rs_y{it}")
                    nc.vector.tensor_mul(yn[0:p, 0:w], y, a[0:p, 0:w])
                    y = yn[0:p, 0:w]

        def red3(a_ap, b_ap, acc_ap, width, scr_tag="red_scr"):
            """acc[:, sh] = sum_k a[:, sh, k]*b[:, sh, k] (merged 3D).

            For width==D the reduction is two-stage (8-wide then 16-wide)
            to keep fp32 summation error well below the 1e-6 epsilon that
            guards the mobius 1-g2 cancellation (serial 128-term sums have
            ~4e-6 error, enough to flip the sign of 1-g2+1e-6 at rows where
            g2 -> 1 and negate whole tanh-saturated H rows)."""
            scr = mid.tile([128, SH, width], FP, name=scr_tag, tag=scr_tag)
            sv = scr[:, :, 0:width]
            nc.vector.tensor_mul(sv, a_ap, b_ap)
            if width == D:
                p1 = mid.tile([128, SH, 16], FP, name=scr_tag + "_p1",
                              tag="red_p1")
                nc.vector.tensor_reduce(
                    p1[:], scr[:].rearrange("p s (a b) -> p s a b", b=8),
                    axis=AX.X, op=OP.add)
                nc.vector.tensor_reduce(acc_ap, p1[:], axis=AX.X, op=OP.add)
            else:
                nc.vector.tensor_reduce(acc_ap, sv, axis=AX.X, op=OP.add)

        def sos_act(src3d, acc_col, width):
            """per-si Square+accum on ACT — used in phases where the scalar
            engine is otherwise idle, to offload the DVE bottleneck."""
            for ih in range(SH):
                scr = mid.tile([128, width], FP, name="sa_scr", tag="sa_scr")
                act(scr[:, 0:width], src3d[:, ih, :], AF.Square,
                    accum=acc_col[:, ih : ih + 1])

        def sos_wide(src_ap, accum_ap):
            """row sum-of-squares for a [128, ~1024] tile."""
            if ACT_ACCUM_OK:
                scr = mid.tile([128, S], FP, name="scr_wide", tag="scr_wide")
                act(scr[:, 0 : src_ap.free_size()], src_ap, AF.Square,
                    accum=accum_ap)
            else:
                scr = mid.tile([128, S], FP, name="scr_wide", tag="scr_wide")
                sv = scr[:, 0 : src_ap.free_size()]
                nc.vector.tensor_mul(sv, src_ap, src_ap)
                nc.vector.tensor_reduce(accum_ap, sv, axis=AX.X, op=OP.add)

        # ================= per-batch program =================
        for b in range(BPC):
            # ---------- phase A: p2l for both sides ----------
            def p2l_side(x_dram_r, side):
                xs = mid.tile([128, SH, D], FP, name=f"x_{side}", tag=f"x_{side}")
                nc.sync.dma_start(xs[:], x_dram_r[b])
                x2 = col(f"x2_{side}")
                red3(xs[:, :, :], xs[:, :, :], x2[:], D)
                # (1 - x2) + 1e-6 in two steps to match the reference's fp32
                # rounding exactly (1-x2 is Sterbenz-exact; the +1e-6 then
                # rounds identically) — this is a catastrophic-cancellation
                # amplifier near the ball boundary.
                t = col(f"pa_t_{side}")
                ts(t[:], x2[:], -1.0, 1.0, OP.mult, OP.add)
                ts(t[:], t[:], 1e-6, None, OP.add)
                inv = col(f"pa_inv_{side}")
                recip(inv[:], t[:])
                t1 = col(f"pa_t1_{side}")
                ts(t1[:], x2[:], 1.0, None, OP.add)
                at = col(f"pa_at_{side}")
                nc.vector.tensor_mul(at[:], t1[:], inv[:])
                ts(at[:], at[:], SQK, None, OP.mult)
                a2 = col(f"pa_a2_{side}")
                ts(a2[:], inv[:], 2.0 * SQK, None, OP.mult)

                lx = mid.tile([128, SH, Dp], FP, name=f"l_{side}", tag=f"l_{side}")
                for sh in range(SH):
                    ts(lx[:, sh, 1:Dp], xs[:, sh, :], a2[:, sh : sh + 1], None,
                       OP.mult)
                nc.vector.tensor_copy(lx[:, :, 0], at[:])

                lT_sp = mid.tile([128, S], FP, name=f"lT_{side}", tag=f"lT_{side}")
                for sh in range(SH):
                    tp = ps_mid.tile([128, 128], FP, name="tp", tag="tp")
                    nc.tensor.transpose(tp[:], lx[:, sh, 1:Dp], ident[:])
                    nc.scalar.copy(lT_sp[:, sh * 128 : (sh + 1) * 128], tp[:])
                # [2, S] k-tile: row 0 = time, row 1 = ones (bias fold),
                # built via [128,2]-pair transposes (base partition 0).
                at2 = mid.tile([128, 2 * SH], FP, name=f"at2_{side}",
                               tag="at2")
                nc.vector.memset(at2[:], 1.0)
                nc.vector.tensor_copy(at2[:, 0 : 2 * SH : 2], at[:])
                lT_tc = mid.tile([2, S], FP, name=f"lTtc_{side}", tag=f"lTtc_{side}")
                for sh in range(SH):
                    tp2 = ps_mid.tile([128, 128], FP, name="tp", tag="tp")
                    nc.tensor.transpose(tp2[0:2, :],
                                        at2[:, 2 * sh : 2 * sh + 2], ident[:])
                    nc.scalar.copy(lT_tc[0:2, sh * 128 : (sh + 1) * 128],
                                   tp2[0:2, :])
                return lx, lT_sp, lT_tc

            ls, lsT_sp, lsT_tc = p2l_side(sr_r, "s")
            lc, lcT_sp, lcT_tc = p2l_side(cr_r, "c")

            # ---------- the three LorentzLinears ----------
            def linear_mm(inT_sp, inT_tc, wname):
                sp_t, tc_t = wsb[wname]
                y = mid.tile([128, SH, Dp], FP, name=f"y_{wname}", tag=f"y_{wname}")
                for ih in range(SH):
                    yp = ps_mid.tile([128, Dp], FP, name="yp", tag="tp")
                    nc.tensor.matmul(yp[:], inT_sp[:, ih * 128 : (ih + 1) * 128],
                                     sp_t[:], start=True, stop=False)
                    nc.tensor.matmul(yp[:], inT_tc[:, ih * 128 : (ih + 1) * 128],
                                     tc_t[:], start=False, stop=True)
                    nc.scalar.copy(y[:, ih, :], yp[:])
                s2 = col(f"s2_{wname}")
                red3(y[:, :, 1:Dp], y[:, :, 1:Dp], s2[:], D)
                y0 = col(f"y0_{wname}")
                nc.vector.tensor_copy(y0[:], y[:, :, 0])
                return y, s2, y0

            y_lw, s2_lw, y0_lw = linear_mm(lcT_sp, lcT_tc, "wl")
            y_ws, s2_ws, y0_ws = linear_mm(lsT_sp, lsT_tc, "ws")
            y_wc, s2_wc, y0_wc = linear_mm(lcT_sp, lcT_tc, "wc")

            # time cols
            def lin_time(y0, sname, scale_val):
                e = col(f"e_{sname}")
                act(e[:], y0[:], AF.Exp, scale=-1.0)
                ts(e[:], e[:], 1.0, None, OP.add)
                r = col(f"r_{sname}")
                recip(r[:], e[:])
                tcol = col(f"t_{sname}")
                ts(tcol[:], r[:], scale_val, 1.1, OP.mult, OP.add)
                return tcol

            t_lw = lin_time(y0_lw, "lw", consts["sl"])
            t_ws = lin_time(y0_ws, "ws", consts["ss"])
            t_wc = lin_time(y0_wc, "wc", consts["sc"])

            # fac = sqrt((t^2-1)/s2) = (t^2-1) * rsqrt((t^2-1)*max(s2,1e-8))
            def lin_fac(tcol, s2, sname):
                s2c = col(f"s2c_{sname}")
                ts(s2c[:], s2[:], 1e-8, None, OP.max)
                t2m1 = col(f"t2m1_{sname}")
                nc.vector.tensor_mul(t2m1[:], tcol[:], tcol[:])
                ts(t2m1[:], t2m1[:], 1.0, None, OP.subtract)
                u = col(f"u_{sname}")
                nc.vector.tensor_mul(u[:], t2m1[:], s2c[:])
                r = col(f"rf_{sname}")
                rsqrt(r[:], u[:])
                fac = col(f"fac_{sname}")
                nc.vector.tensor_mul(fac[:], t2m1[:], r[:])
                return fac

            fac_lw = lin_fac(t_lw, s2_lw, "lw")
            fac_ws = lin_fac(t_ws, s2_ws, "ws")
            fac_wc = lin_fac(t_wc, s2_wc, "wc")

            # Lw row-major + transposed
            lw = mid.tile([128, SH, Dp], FP, name="lw", tag="lw")
            for ih in range(SH):
                ts(lw[:, ih, 1:Dp], y_lw[:, ih, 1:Dp], fac_lw[:, ih : ih + 1],
                   None, OP.mult)
            nc.vector.tensor_copy(lw[:, :, 0], t_lw[:])
            lwT_sp = mid.tile([128, S], FP, name="lwT_sp", tag="lwT_sp")
            for ih in range(SH):
                tp = ps_mid.tile([128, 128], FP, name="tp", tag="tp")
                nc.tensor.transpose(tp[:], lw[:, ih, 1:Dp], ident[:])
                nc.scalar.copy(lwT_sp[:, ih * 128 : (ih + 1) * 128], tp[:])
            # [2, S] k-tile for L0/L0T: row 0 = Lw time, row 1 = zeros.
            lw2 = mid.tile([128, 2 * SH], FP, name="lw2", tag="at2")
            nc.vector.memset(lw2[:], 0.0)
            nc.vector.tensor_copy(lw2[:, 0 : 2 * SH : 2], t_lw[:])
            lwT_tc = mid.tile([2, S], FP, name="lwT_tc", tag="lwT_tc")
            for ih in range(SH):
                tp2 = ps_mid.tile([128, 128], FP, name="tp", tag="tp")
                nc.tensor.transpose(tp2[0:2, :],
                                    lw2[:, 2 * ih : 2 * ih + 2], ident[:])
                nc.scalar.copy(lwT_tc[0:2, ih * 128 : (ih + 1) * 128],
                               tp2[0:2, :])

            # Ps / Pc  (l2p of the ws/wc linears)
            def build_p(y, tcol, fac, sname):
                den = col(f"pden_{sname}")
                ts(den[:], tcol[:], SQK, None, OP.add)
                idn = col(f"pidn_{sname}")
                recip(idn[:], den[:])
                pf = col(f"pf_{sname}")
                nc.vector.tensor_mul(pf[:], fac[:], idn[:])
                p = mid.tile([128, SH, D], FP, name=f"p_{sname}", tag=f"p_{sname}")
                for ih in range(SH):
                    ts(p[:, ih, :], y[:, ih, 1:Dp], pf[:, ih : ih + 1], None,
                       OP.mult)
                return p

            Ps = build_p(y_ws, t_ws, fac_ws, "s")
            Pc = build_p(y_wc, t_wc, fac_wc, "c")

            # ---------- L0 / L (row-major), L0T / LT (col-major) ----------
            L = big.tile([128, SH, S], FP, name="L", tag="L")
            ss = col("ss")
            for si in range(SH):
                pl = ps_big.tile([128, S], FP, name="pl", tag="pl")
                for ch in range(2):
                    csl = slice(ch * 512, (ch + 1) * 512)
                    nc.tensor.matmul(pl[:, csl],
                                     lsT_sp[:, si * 128 : (si + 1) * 128],
                                     lwT_sp[:, csl], start=True, stop=False)
                    nc.tensor.matmul(pl[:, csl],
                                     lsT_tc[0:2, si * 128 : (si + 1) * 128],
                                     lwT_tc[0:2, csl], start=False, stop=True)
                act(L[:, si, 1:S], pl[:, 1:S], AF.Tanh)
                sos_wide(L[:, si, 1:S], ss[:, si : si + 1])

            timeL = col("timeL")
            u_tl = col("u_tl")
            ts(u_tl[:], ss[:], 1.0, None, OP.add)
            r_tl = col("r_tl")
            rsqrt(r_tl[:], u_tl[:])
            nc.vector.tensor_mul(timeL[:], u_tl[:], r_tl[:])
            nc.vector.tensor_copy(L[:, :, 0], timeL[:])
            ln2 = col("ln2")
            ts(ln2[:], ss[:], 2.0, 1.0, OP.mult, OP.add)
            invLn = col("invLn")
            rsqrt(invLn[:], ln2[:])
            lnfac = col("lnfac")
            ts(lnfac[:], invLn[:], C_ART, None, OP.mult)

            LT = big.tile([128, SH, S], FP, name="LT", tag="LT")
            for ci in range(SH):
                pl = ps_big.tile([128, S], FP, name="pl", tag="pl")
                for ch in range(2):
                    csl = slice(ch * 512, (ch + 1) * 512)
                    nc.tensor.matmul(pl[:, csl],
                                     lwT_sp[:, ci * 128 : (ci + 1) * 128],
                                     lsT_sp[:, csl], start=True, stop=False)
                    nc.tensor.matmul(pl[:, csl],
                                     lwT_tc[0:2, ci * 128 : (ci + 1) * 128],
                                     lsT_tc[0:2, csl], start=False, stop=True)
                nc.scalar.activation(LT[:, ci, :], pl[:, :], AF.Tanh)
            # overwrite LT row c=0 with timeL (single-column transposes)
            for sh in range(SH):
                tpt = ps_mid.tile([128, 128], FP, name="tp", tag="tp")
                nc.tensor.transpose(tpt[0:1, :], timeL[:, sh : sh + 1],
                                    ident[:])
                nc.scalar.copy(LT[0:1, 0, sh * 128 : (sh + 1) * 128],
                               tpt[0:1, :])
            ltn2 = col("ltn2")
            for ci in range(SH):
                sos_wide(LT[:, ci, :], ltn2[:, ci : ci + 1])
            invLTn = col("invLTn")
            rsqrt(invLTn[:], ltn2[:])
            ltnfac = col("ltnfac")
            ts(ltnfac[:], invLTn[:], C_ART, None, OP.mult)

            # ---------- attention sides ----------
            def attn_side(Lmat, P_self, P_other, lfac, l_self, wh_sb, out_r,
                          side):
                o1 = mid.tile([128, SH, D], FP, name=f"o1_{side}",
                              tag=f"o1_{side}")
                for si in range(SH):
                    po = ps_mid.tile([128, 128], FP, name="po", tag="tp")
                    for ci in range(SH):
                        nc.tensor.matmul(po[:],
                                         Lmat[:, ci, si * 128 : (si + 1) * 128],
                                         P_other[:, ci, :],
                                         start=(ci == 0), stop=(ci == SH - 1))
                    nc.scalar.copy(o1[:, si, :], po[:])
                mxn2 = col(f"mxn2_{side}")
                red3(o1[:, :, :], o1[:, :, :], mxn2[:], D)
                rmx = col(f"rmx_{side}")
                rsqrt(rmx[:], mxn2[:])           # rmx = 1/mxn (inf if mxn=0)
                arg = col(f"arg_{side}")
                nc.vector.tensor_mul(arg[:], mxn2[:], rmx[:])   # = mxn
                nc.vector.tensor_mul(arg[:], arg[:], lfac[:])
                th = col(f"th_{side}")
                act(th[:], arg[:], AF.Tanh)
                g = col(f"g_{side}")
                nc.vector.tensor_mul(g[:], th[:], rmx[:])
                xyp = col(f"xyp_{side}")
                red3(P_self[:, :, :], o1[:, :, :], xyp[:], D)
                x2p = col(f"x2p_{side}")
                red3(P_self[:, :, :], P_self[:, :, :], x2p[:], D)
                xy = col(f"xy_{side}")
                nc.vector.tensor_mul(xy[:], xyp[:], g[:])
                y2 = col(f"y2_{side}")
                nc.vector.tensor_mul(y2[:], th[:], th[:])   # g^2*mxn^2 == th^2
                t12 = col(f"t12_{side}")
                ts(t12[:], xy[:], 2.0, 1.0, OP.mult, OP.add)   # 1+2xy
                cs_ = col(f"cs_{side}")
                nc.vector.tensor_add(cs_[:], t12[:], y2[:])    # 1+2xy+y2
                cy = col(f"cy_{side}")
                ts(cy[:], x2p[:], -1.0, 1.0, OP.mult, OP.add)  # 1-x2
                x2y2 = col(f"x2y2_{side}")
                nc.vector.tensor_mul(x2y2[:], x2p[:], y2[:])
                den = col(f"den_{side}")
                nc.vector.tensor_add(den[:], t12[:], x2y2[:])
                ts(den[:], den[:], MINN, None, OP.max)
                iden = col(f"iden_{side}")
                recip(iden[:], den[:])
                csp = col(f"csp_{side}")
                nc.vector.tensor_mul(csp[:], cs_[:], iden[:])
                cyg = col(f"cyg_{side}")
                nc.vector.tensor_mul(cyg[:], cy[:], iden[:])
                nc.vector.tensor_mul(cyg[:], cyg[:], g[:])
                # G = csp*P_self + cyg*o1     (mobius_add result)
                G = mid.tile([128, SH, D], FP, name=f"G_{side}", tag=f"G_{side}")
                for si in range(SH):
                    t1 = mid.tile([128, D], FP, name="gt1", tag="gt1")
                    ts(t1[:], P_self[:, si, :], csp[:, si : si + 1], None,
                       OP.mult)
                    t2 = mid.tile([128, D], FP, name="gt2", tag="gt2")
                    ts(t2[:], o1[:, si, :], cyg[:, si : si + 1], None, OP.mult)
                    nc.vector.tensor_add(G[:, si, :], t1[:], t2[:])
                g2 = col(f"g2_{side}")
                red3(G[:, :, :], G[:, :, :], g2[:], D)
                # two-step (1-g2)+1e-6: see p2l comment (worst amplifier:
                # a = sqrt(k)/(1-g2+1e-6) with g2 close to 1)
                d2 = col(f"d2_{side}")
                ts(d2[:], g2[:], -1.0, 1.0, OP.mult, OP.add)
                ts(d2[:], d2[:], 1e-6, None, OP.add)
                ia = col(f"ia_{side}")
                recip(ia[:], d2[:])
                a2 = col(f"a2_{side}")
                ts(a2[:], ia[:], 2.0 * SQK, None, OP.mult)
                # H = [sqrt(1+th2) | tanh(a2 * G)]
                H = mid.tile([128, SH, Dp], FP, name=f"H_{side}",
                             tag=f"H_{side}")
                for si in range(SH):
                    act(H[:, si, 1:Dp], G[:, si, :], AF.Tanh,
                        scale=a2[:, si : si + 1])
                th2 = col(f"th2_{side}")
                red3(H[:, :, 1:Dp], H[:, :, 1:Dp], th2[:], D)
                u_th = col(f"u_th_{side}")
                ts(u_th[:], th2[:], 1.0, None, OP.add)
                r_th = col(f"r_th_{side}")
                rsqrt(r_th[:], u_th[:])
                timeh = col(f"timeh_{side}")
                nc.vector.tensor_mul(timeh[:], u_th[:], r_th[:])
                nc.vector.tensor_copy(H[:, :, 0], timeh[:])
                hn2 = col(f"hn2_{side}")
                nc.vector.tensor_add(hn2[:], u_th[:], th2[:])   # 1+2*th2
                invHn = col(f"invHn_{side}")
                rsqrt(invHn[:], hn2[:])
                wfac = col(f"wfac_{side}")
                ts(wfac[:], invHn[:], C_ART, None, OP.mult)
                # w = tanh(<H, wh> * wfac);  A = softmax(w)
                mx = col(f"mx_{side}")
                red3(H[:, :, :], wh_sb[:].rearrange("p (sh d) -> p sh d", sh=SH),
                     mx[:], Dp, scr_tag="red_scr_dp")
                warg = col(f"warg_{side}")
                nc.vector.tensor_mul(warg[:], mx[:], wfac[:])
                wcol = col(f"w_{side}")
                act(wcol[:], warg[:], AF.Tanh)
                e = col(f"esm_{side}")
                act(e[:], wcol[:], AF.Exp)
                pz = ps_sm.tile([1, SH], FP, name="pz", tag="sm")
                nc.tensor.matmul(pz[:], ones_col[:], e[:])
                z = cols.tile([1, 1], FP, name=f"z_{side}", tag=f"z_{side}")
                nc.vector.reduce_sum(z[:], pz[:], axis=AX.X)
                iz = cols.tile([1, 1], FP, name=f"iz_{side}", tag=f"iz_{side}")
                recip(iz[:], z[:])
                pb = ps_sm.tile([128, 1], FP, name="pb", tag="sm")
                nc.tensor.matmul(pb[:], ones_row[:], iz[:])
                izb = cols.tile([128, 1], FP, name=f"izb_{side}",
                                tag=f"izb_{side}")
                nc.scalar.copy(izb[:], pb[:])
                acol = col(f"acol_{side}")
                ts(acol[:], e[:], izb[:, 0:1], None, OP.mult)
                nc.sync.dma_start(out_r[b], acol[:])
                # centroid
                pC = ps_sm.tile([128, 1], FP, name="pC", tag="sm")
                for sh in range(SH):
                    nc.tensor.matmul(pC[:], l_self[:, sh, 1:Dp],
                                     acol[:, sh : sh + 1],
                                     start=(sh == 0), stop=(sh == SH - 1))
                pT = ps_sm.tile([1, 1], FP, name="pT", tag="sm")
                for sh in range(SH):
                    nc.tensor.matmul(pT[:], l_self[:, sh, 0:1],
                                     acol[:, sh : sh + 1],
                                     start=(sh == 0), stop=(sh == SH - 1))
                cS = cols.tile([128, 1], FP, name=f"cS_{side}",
                               tag=f"cS_{side}")
                nc.scalar.copy(cS[:], pC[:])
                tS = cols.tile([1, 1], FP, name=f"tS_{side}", tag=f"tS_{side}")
                nc.scalar.copy(tS[:], pT[:])
                cSq = cols.tile([128, 1], FP, name=f"cSq_{side}",
                                tag=f"cSq_{side}")
                act(cSq[:], cS[:], AF.Square)
                pS = ps_sm.tile([1, 1], FP, name="pS", tag="sm")
                nc.tensor.matmul(pS[:], cSq[:], ones_col[:])
                sp2 = cols.tile([1, 1], FP, name=f"sp2_{side}",
                                tag=f"sp2_{side}")
                nc.scalar.copy(sp2[:], pS[:])
                tsq2 = cols.tile([1, 1], FP, name=f"tsq_{side}",
                                 tag=f"tsq_{side}")
                act(tsq2[:], tS[:], AF.Square)
                inner = cols.tile([1, 1], FP, name=f"inner_{side}",
                                  tag=f"inner_{side}")
                nc.vector.tensor_sub(inner[:], sp2[:], tsq2[:])
                iabs = cols.tile([1, 1], FP, name=f"iabs_{side}",
                                 tag=f"iabs_{side}")
                act(iabs[:], inner[:], AF.Abs)
                ts(iabs[:], iabs[:], 1e-8, None, OP.max)
                rinv = cols.tile([1, 1], FP, name=f"rinv_{side}",
                                 tag=f"rinv_{side}")
                rsqrt(rinv[:], iabs[:], p=1, w=1)
                pB2 = ps_sm.tile([128, 1], FP, name="pB2", tag="sm")
                nc.tensor.matmul(pB2[:], ones_row[:], rinv[:])
                rb = cols.tile([128, 1], FP, name=f"rb_{side}",
                               tag=f"rb_{side}")
                nc.scalar.copy(rb[:], pB2[:])
                csp_out = cols.tile([128, 1], FP, name=f"cspo_{side}",
                                    tag=f"cspo_{side}")
                ts(csp_out[:], cS[:], rb[:, 0:1], None, OP.mult)
                r2 = cols.tile([1, 1], FP, name=f"r2_{side}", tag=f"r2_{side}")
                nc.vector.tensor_mul(r2[:], rinv[:], rinv[:])
                spn = cols.tile([1, 1], FP, name=f"spn_{side}",
                                tag=f"spn_{side}")
                nc.vector.tensor_mul(spn[:], sp2[:], r2[:])
                return csp_out, spn

            co_s, spn_s = attn_side(LT, Ps, Pc, lnfac, ls, whs_sb, as_r, "s")
            co_c, spn_c = attn_side(L, Pc, Ps, ltnfac, lc, whc_sb, ac_r, "c")

            # ---------- final assembly ----------
            tsum = cols.tile([1, 1], FP, name="tsum", tag="tsum")
            nc.vector.tensor_add(tsum[:], spn_s[:], spn_c[:])
            ts(tsum[:], tsum[:], 1.0, None, OP.add)
            rt = cols.tile([1, 1], FP, name="rt", tag="rt")
            rsqrt(rt[:], tsum[:], p=1, w=1)
            tfin = cols.tile([1, 1], FP, name="tfin", tag="tfin")
            nc.vector.tensor_mul(tfin[:], tsum[:], rt[:])
            nc.sync.dma_start(co_r[b, 0:1, :], tfin[:])
            nc.sync.dma_start(co_r[b, 1 : 1 + D, :], co_s[:])
            nc.sync.dma_start(co_r[b, 1 + D : 1 + 2 * D, :], co_c[:])

    nc.compile()
    return nc


_CACHE = {}
LAST_RES = None


def kernel(sentence_rep, comment_rep, Wl_w, Wl_b, Wl_scale, Ws_w, Ws_b,
           Ws_scale, Wc_w, Wc_b, Wc_scale, whs, whc):
    sentence_rep = np.ascontiguousarray(sentence_rep, np.float32)
    comment_rep = np.ascontiguousarray(comment_rep, np.float32)

    consts = {
        "sl": float(np.exp(np.float32(Wl_scale))),
        "ss": float(np.exp(np.float32(Ws_scale))),
        "sc": float(np.exp(np.float32(Wc_scale))),
    }
    key = tuple(sorted(consts.items()))
    if key not in _CACHE:
        _CACHE[key] = _build(consts)
    nc = _CACHE[key]

    def prep_w(W, b_):
        WT = np.ascontiguousarray(np.asarray(W, np.float32).T)  # [din, dout]
        sp = np.ascontiguousarray(WT[1:129])                    # [128, 129]
        tc_ = np.ascontiguousarray(
            np.stack([WT[0], np.asarray(b_, np.float32)]))      # [2, 129]
        return sp, tc_

    wl_sp, wl_tc = prep_w(Wl_w, Wl_b)
    ws_sp, ws_tc = prep_w(Ws_w, Ws_b)
    wc_sp, wc_tc = prep_w(Wc_w, Wc_b)
    whs_b = np.ascontiguousarray(np.broadcast_to(
        np.asarray(whs, np.float32).reshape(1, Dp), (128, SH, Dp)
    ).reshape(128, SH * Dp) if False else np.tile(
        np.asarray(whs, np.float32).reshape(1, 1, Dp), (128, SH, 1)
    ).reshape(128, SH * Dp))
    whc_b = np.ascontiguousarray(np.tile(
        np.asarray(whc, np.float32).reshape(1, 1, Dp), (128, SH, 1)
    ).reshape(128, SH * Dp))

    in_maps = []
    for c in range(NCORES):
        sl_ = slice(c * BPC, (c + 1) * BPC)
        in_maps.append({
            "sr": np.ascontiguousarray(sentence_rep[sl_]),
            "cr": np.ascontiguousarray(comment_rep[sl_]),
            "wl_sp": wl_sp, "wl_tc": wl_tc,
            "ws_sp": ws_sp, "ws_tc": ws_tc,
            "wc_sp": wc_sp, "wc_tc": wc_tc,
            "whs_b": whs_b, "whc_b": whc_b,
            "ident_in": np.eye(128, dtype=np.float32),
        })

    res = run_bass_kernel_spmd(nc, in_maps, core_ids=list(range(NCORES)))
    global LAST_RES
    LAST_RES = res

    co = np.concatenate([r["co_out"] for r in res.results], 0)
    As = np.concatenate([r["as_out"] for r in res.results], 0)[:, None, :]
    Ac = np.concatenate([r["ac_out"] for r in res.results], 0)[:, None, :]
    return co, As, Ac
